# revision 7
# baseline (speedup 1.0000x reference)
"""Barycentric interpolation kernel for Trainium2 (8 NeuronCores).

Problem: out[b, m] = sum_k f_values[b, tri_idx[m, k]] * bary_weights[m, k]
  f_values: (128, 10000) fp32, tri_idx: (500000, 3) int32, bary: (500000, 3) fp32
  out: (128, 500000) fp32

Strategy (per core, M sharded 8 ways, batch b on the 128 SBUF partitions):
  - f_values (128, 10000) fp32 stays resident in SBUF.
  - For each m-tile of T=2048 target points, one GPSIMD ap_gather with the
    tile's 3*T vertex indices (int16, wrapped per 16-partition group)
    produces G (128_batch x 3T) fp32 on-chip: G[b, 3m+k] = f[b, idx[m, k]].
    The index list is laid out k-planar: [idx_k0 | idx_k1 | idx_k2].
  - Weights w_k are broadcast across the 128 batch partitions with a
    rank-1 (K=1) TensorE matmul into PSUM; VectorE reads them from PSUM.
  - VectorE: 3x tensor_mul (fp32 x PSUM fp32) + 2x tensor_add, all fp32.
  - fp32 result tile DMA'd to HBM.
"""

import numpy as np
from contextlib import ExitStack

B = 128
N = 10000
M = 500000
NCORES = 8
T = 2048                       # target points per tile
TILES_PER_CORE = 31
M_LOC = T * TILES_PER_CORE     # 63488 padded points per core
M_PAD = M_LOC * NCORES         # 507904


def _split_drain_waits(nc, mybir):
    """walrus in this toolchain accepts at most one sync-wait on InstDrain;
    move extra waits onto no-ops inserted right after the drain."""
    for f in nc.m.functions:
        for bb in f.blocks:
            insts = list(bb.instructions)
            out, changed = [], False
            for inst in insts:
                out.append(inst)
                si = inst.sync_info
                if (
                    type(inst).__name__ == "InstDrain"
                    and si is not None
                    and si.on_wait is not None
                    and len(si.on_wait) > 1
                ):
                    extras = list(si.on_wait[1:])
                    si.on_wait = [si.on_wait[0]]
                    for w in extras:
                        out.append(
                            mybir.InstNoOp(
                                name=nc.get_next_instruction_name(),
                                engine=inst.engine,
                                sync_info=mybir.SyncInfo(on_wait=[w], on_update=[]),
                                bass_nofuse=True,
                            )
                        )
                    changed = True
            if changed:
                bb.instructions = out


def build_nc(tiles_per_core=TILES_PER_CORE, t=T):
    import concourse.bacc as bacc
    import concourse.tile as tile
    import concourse.mybir as mybir

    fp16 = mybir.dt.float16
    fp32 = mybir.dt.float32
    i16 = mybir.dt.int16

    nc = bacc.Bacc()
    f_d = nc.declare_dram_parameter("f", [128, N], fp32, isOutput=False)
    idx = nc.declare_dram_parameter(
        "idx", [tiles_per_core, 128, 3 * (t // 16)], i16, isOutput=False
    )
    wts = nc.declare_dram_parameter("wts", [tiles_per_core, 3 * t], fp32, isOutput=False)
    ones = nc.declare_dram_parameter("ones", [1, 128], fp32, isOutput=False)
    out = nc.declare_dram_parameter(
        "out", [128, tiles_per_core * t], fp32, isOutput=True
    )

    with ExitStack() as ctx:
        tc = ctx.enter_context(tile.TileContext(nc))
        cpool = ctx.enter_context(tc.tile_pool(name="const", bufs=1))
        f_sb = cpool.tile([128, N, 1], fp32)
        nc.sync.dma_start(f_sb[:, :, 0], f_d[:])
        ones_sb = cpool.tile([1, 128], fp32)
        nc.sync.dma_start(ones_sb[:], ones[:])

        idxp = ctx.enter_context(tc.tile_pool(name="idx", bufs=3))
        wp = ctx.enter_context(tc.tile_pool(name="w", bufs=2))
        gp = ctx.enter_context(tc.tile_pool(name="g", bufs=2))
        pp = ctx.enter_context(tc.tile_pool(name="p", bufs=2))
        op = ctx.enter_context(tc.tile_pool(name="o", bufs=2))
        psp = ctx.enter_context(tc.tile_pool(name="ps", bufs=2, space="PSUM"))

        for ti in range(tiles_per_core):
            idx_sb = idxp.tile([128, 3 * (t // 16)], i16, tag="idx")
            nc.sync.dma_start(idx_sb[:], idx[ti])
            w_sb = wp.tile([1, 3 * t], fp32, tag="w")
            nc.sync.dma_start(w_sb[:], wts[ti][None, :])

            g3 = gp.tile([128, 3 * t, 1], fp32, tag="g3")
            nc.gpsimd.ap_gather(
                g3[:], f_sb[:], idx_sb[:],
                channels=128, num_elems=N, d=1, num_idxs=3 * t,
            )

            outt = op.tile([128, t], fp32, tag="o")
            s01 = op.tile([128, t], fp32, tag="s01")
            for k in range(3):
                ps = psp.tile([128, t], fp32, tag="ps")
                for c in range(t // 512):
                    nc.tensor.matmul(
                        ps[:, c * 512 : (c + 1) * 512],
                        ones_sb[:],
                        w_sb[:, k * t + c * 512 : k * t + (c + 1) * 512],
                        start=True,
                        stop=True,
                    )
                pk = pp.tile([128, t], fp32, tag="p")
                nc.vector.tensor_mul(pk[:], g3[:, k * t : (k + 1) * t, 0], ps[:])
                if k == 1:
                    nc.vector.tensor_add(s01[:], prev_pk[:], pk[:])  # noqa: F821
                elif k == 2:
                    nc.vector.tensor_add(outt[:], s01[:], pk[:])
                prev_pk = pk
            nc.sync.dma_start(out[:, ti * t : (ti + 1) * t], outt[:])

    nc.finalize()
    _split_drain_waits(nc, mybir)
    return nc


# ---------------------------------------------------------------- host side --


def _prep_core_inputs(ti_core, w_core, tiles_per_core=TILES_PER_CORE, t=T):
    """ti_core: (M_LOC, 3) int, w_core: (M_LOC, 3) float -> idx/wts host arrays."""
    s = t // 16
    # idx: (tiles, 128, 3*S) int16, k-planar, wrapped in 16 partitions, repl x8
    a = ti_core.reshape(tiles_per_core, t, 3).astype(np.int16)
    a = a.transpose(0, 2, 1).reshape(tiles_per_core, 3, s, 16)  # [tile, k, s, p]
    a = a.transpose(0, 3, 1, 2).reshape(tiles_per_core, 16, 3 * s)
    idx = np.ascontiguousarray(np.tile(a, (1, 8, 1)))  # (tiles, 128, 3*S)
    # wts: (tiles, 3*t) fp32 planar per k
    w = w_core.reshape(tiles_per_core, t, 3).astype(np.float32)
    wts = np.ascontiguousarray(w.transpose(0, 2, 1).reshape(tiles_per_core, 3 * t))
    return idx, wts


def kernel(f_values, tri_idx, bary_weights):
    from concourse.bass_utils import run_bass_kernel_spmd

    f_values = np.ascontiguousarray(np.asarray(f_values, dtype=np.float32))
    tri_idx = np.asarray(tri_idx)
    bary_weights = np.asarray(bary_weights)

    ti = np.zeros((M_PAD, 3), np.int32)
    ti[:M] = tri_idx
    w = np.zeros((M_PAD, 3), np.float32)
    w[:M] = bary_weights

    ones = np.ones((1, 128), np.float32)
    in_maps = []
    for c in range(NCORES):
        sl = slice(c * M_LOC, (c + 1) * M_LOC)
        idx_h, wts_h = _prep_core_inputs(ti[sl], w[sl])
        in_maps.append({"f": f_values, "idx": idx_h, "wts": wts_h, "ones": ones})

    nc = build_nc()
    res = run_bass_kernel_spmd(nc, in_maps, core_ids=list(range(NCORES)))
    out = np.concatenate([res.results[c]["out"] for c in range(NCORES)], axis=1)
    return np.ascontiguousarray(out[:, :M])


if __name__ == "__main__":
    rng = np.random.default_rng(0)
    f = rng.standard_normal((B, N), dtype=np.float32)
    t_idx = rng.integers(0, N, size=(M, 3)).astype(np.int32)
    bw = rng.random((M, 3), dtype=np.float32)
    bw /= bw.sum(1, keepdims=True)
    got = kernel(f, t_idx, bw)
    exp = np.einsum("bmk,mk->bm", f[:, t_idx], bw)
    err = np.abs(got - exp).max() / np.abs(exp).max()
    print("rel err:", err)


# revision 14
# speedup vs baseline: 1.0456x; 1.0456x over previous
"""Barycentric interpolation kernel for Trainium2 (8 NeuronCores).

Problem: out[b, m] = sum_k f_values[b, tri_idx[m, k]] * bary_weights[m, k]
  f_values: (128, 10000) fp32, tri_idx: (500000, 3) int32, bary: (500000, 3) fp32
  out: (128, 500000) fp32

Strategy (per core, M sharded 8 ways):
  - Batches are packed in fp16 PAIRS: SBUF partition p holds batches (2p, 2p+1)
    as one 32-bit lane value, so 64 partitions cover all 128 batches. f is
    duplicated into partitions 64-127, and the two GPSIMD core groups
    (Q7 cores 0-3 = partitions 0-63, cores 4-7 = 64-127) are given DIFFERENT
    index lists — each core iterates only half the indices per tile, halving
    gather time (GPSIMD is the bottleneck engine).
  - Per m-tile of T=2048 points: partitions 0-63 gather the 3*1024 indices of
    the first 1024 points (k-planar), partitions 64-127 the second 1024.
  - Weights are broadcast to the right partition halves with one K=2 TensorE
    matmul per 512-chunk (lhsT = [half masks], rhs = [wA_dup; wB_dup]) into
    PSUM, then ScalarE casts fp32->fp16 into SBUF.
  - VectorE: 3x tensor_mul + 2x tensor_add, fp16 at 2x perf mode.
  - fp16 result (batch-pair interleaved) is de-interleaved by the store DMA
    access pattern; host upcasts to fp32.
"""

import numpy as np
from contextlib import ExitStack

B = 128
N = 10000
M = 500000
NCORES = 8
T = 2048                       # target points per tile
H = T // 2                     # points per partition-half
TILES_PER_CORE = 31
M_LOC = T * TILES_PER_CORE     # 63488 padded points per core
M_PAD = M_LOC * NCORES         # 507904


def _split_drain_waits(nc, mybir):
    """walrus in this toolchain accepts at most one sync-wait on InstDrain;
    move extra waits onto no-ops inserted right after the drain."""
    for f in nc.m.functions:
        for bb in f.blocks:
            insts = list(bb.instructions)
            out, changed = [], False
            for inst in insts:
                out.append(inst)
                si = inst.sync_info
                if (
                    type(inst).__name__ == "InstDrain"
                    and si is not None
                    and si.on_wait is not None
                    and len(si.on_wait) > 1
                ):
                    extras = list(si.on_wait[1:])
                    si.on_wait = [si.on_wait[0]]
                    for w in extras:
                        out.append(
                            mybir.InstNoOp(
                                name=nc.get_next_instruction_name(),
                                engine=inst.engine,
                                sync_info=mybir.SyncInfo(on_wait=[w], on_update=[]),
                                bass_nofuse=True,
                            )
                        )
                    changed = True
            if changed:
                bb.instructions = out


def build_nc(tiles_per_core=TILES_PER_CORE, t=T):
    import concourse.bacc as bacc
    import concourse.tile as tile
    import concourse.mybir as mybir

    fp16 = mybir.dt.float16
    fp32 = mybir.dt.float32
    i16 = mybir.dt.int16

    h = t // 2          # points per half
    ni = 3 * h          # gather indices per core group per tile
    nc = bacc.Bacc()
    f_d = nc.declare_dram_parameter("f", [128, N], fp32, isOutput=False)
    idx = nc.declare_dram_parameter(
        "idx", [tiles_per_core, 128, ni // 16], i16, isOutput=False
    )
    wts = nc.declare_dram_parameter("wts", [tiles_per_core, 2, 3 * t], fp32,
                                    isOutput=False)
    masks = nc.declare_dram_parameter("masks", [2, 128], fp32, isOutput=False)
    out = nc.declare_dram_parameter(
        "out", [128, tiles_per_core * t], fp16, isOutput=True
    )
    # dst view for de-interleaving stores: row-pair p, batch-parity e, column m
    out_r = out[:].rearrange("(p e) m -> p e m", e=2)

    with ExitStack() as ctx:
        tc = ctx.enter_context(tile.TileContext(nc))
        cpool = ctx.enter_context(tc.tile_pool(name="const", bufs=1))
        f_sb = cpool.tile([128, N, 1], fp32)
        nc.sync.dma_start(f_sb[:, :, 0], f_d[:])
        masks_sb = cpool.tile([2, 128], fp32)
        nc.sync.dma_start(masks_sb[:], masks[:])

        idxp = ctx.enter_context(tc.tile_pool(name="idx", bufs=3))
        wp = ctx.enter_context(tc.tile_pool(name="w", bufs=2))
        gp = ctx.enter_context(tc.tile_pool(name="g", bufs=2))
        wbp = ctx.enter_context(tc.tile_pool(name="wb", bufs=2))
        pp = ctx.enter_context(tc.tile_pool(name="p", bufs=2))
        # (per-tag slots: p0/p1/p2 and wb get independent double-buffers)
        op = ctx.enter_context(tc.tile_pool(name="o", bufs=2))
        psp = ctx.enter_context(tc.tile_pool(name="ps", bufs=4, space="PSUM"))

        for ti in range(tiles_per_core):
            idx_sb = idxp.tile([128, ni // 16], i16, tag="idx")
            nc.sync.dma_start(idx_sb[:], idx[ti])
            w_sb = wp.tile([2, 3 * t], fp32, tag="w")
            nc.sync.dma_start(w_sb[:], wts[ti])

            g3 = gp.tile([128, ni, 1], fp32, tag="g3")
            nc.gpsimd.ap_gather(
                g3[:], f_sb[:], idx_sb[:],
                channels=128, num_elems=N, d=1, num_idxs=ni,
            )
            g16 = g3[:, :, 0].bitcast(fp16)  # [128, 2*ni] batch-pair interleaved

            oute = [
                op.tile([128, h], fp16, name=f"oute{e}", tag=f"o{e}")
                for e in range(2)
            ]
            s01 = op.tile([128, t], fp16, tag="s01")
            for k in range(3):
                wb = wbp.tile([128, t], fp16, tag=f"wb{k}")
                for cc in range(2):
                    ps = psp.tile([128, t // 2], fp32, tag="ps")
                    for c in range(2):
                        off = k * t + cc * (t // 2) + c * 512
                        nc.tensor.matmul(
                            ps[:, c * 512 : (c + 1) * 512],
                            masks_sb[:],
                            w_sb[:, off : off + 512],
                            start=True,
                            stop=True,
                        )
                    nc.scalar.copy(wb[:, cc * (t // 2) : (cc + 1) * (t // 2)], ps[:])
                pk = pp.tile([128, t], fp16, tag=f"p{k}")
                nc.vector.tensor_mul(pk[:], g16[:, k * t : (k + 1) * t], wb[:])
                if k == 1:
                    nc.vector.tensor_add(s01[:], prev_pk[:], pk[:])  # noqa: F821
                elif k == 2:
                    # final add doubles as the batch-pair de-interleave:
                    # one strided add per parity, contiguous outputs
                    s3 = s01[:].rearrange("p (m e) -> p m e", e=2)
                    p3 = pk[:].rearrange("p (m e) -> p m e", e=2)
                    for e in range(2):
                        nc.vector.tensor_add(oute[e][:], s3[:, :, e], p3[:, :, e])
                prev_pk = pk
            # half hf covers points [ti*t + hf*h, +h)
            for hf in range(2):
                for e in range(2):
                    nc.sync.dma_start(
                        out_r[:, e, ti * t + hf * h : ti * t + (hf + 1) * h],
                        oute[e][64 * hf : 64 * (hf + 1)],
                    )

    nc.finalize()
    _split_drain_waits(nc, mybir)
    return nc


# ---------------------------------------------------------------- host side --


def _prep_f(f_values):
    """(128, N) fp32 -> (128, N) fp32-viewed fp16 batch pairs, duplicated."""
    f16 = f_values.astype(np.float16)                    # (128, N)
    pk = np.empty((64, N, 2), np.float16)
    pk[:, :, 0] = f16[0::2]
    pk[:, :, 1] = f16[1::2]
    packed = pk.reshape(64, 2 * N).view(np.float32)      # (64, N)
    return np.ascontiguousarray(np.concatenate([packed, packed], axis=0))


def _wrap16(lists):
    """(G, n) index lists -> (G, 16, n//16) wrapped: idx j at row j%16."""
    g, n = lists.shape
    return lists.reshape(g, n // 16, 16).transpose(0, 2, 1)


def _prep_core_inputs(ti_core, w_core, tiles_per_core=TILES_PER_CORE, t=T):
    h = t // 2
    ni = 3 * h
    # per tile, half A = points [0, h), half B = [h, t); k-planar lists
    a = ti_core.reshape(tiles_per_core, 2, h, 3).astype(np.int16)
    lists = a.transpose(0, 1, 3, 2).reshape(tiles_per_core * 2, ni)  # [tile*half, 3h]
    wrapped = _wrap16(lists).reshape(tiles_per_core, 2, 16, ni // 16)
    idx = np.empty((tiles_per_core, 128, ni // 16), np.int16)
    idx[:, :64] = np.tile(wrapped[:, 0], (1, 4, 1))
    idx[:, 64:] = np.tile(wrapped[:, 1], (1, 4, 1))
    # wts: (tiles, 2, 3t): row h = k-planar weights of half h, each dup x2
    w = w_core.reshape(tiles_per_core, 2, h, 3).astype(np.float32)
    w = w.transpose(0, 1, 3, 2)                          # [tile, half, k, m]
    wts = np.repeat(w.reshape(tiles_per_core, 2, 3 * h), 2, axis=-1)
    return np.ascontiguousarray(idx), np.ascontiguousarray(wts)


def kernel(f_values, tri_idx, bary_weights):
    from concourse.bass_utils import run_bass_kernel_spmd

    f_values = np.ascontiguousarray(np.asarray(f_values, dtype=np.float32))
    tri_idx = np.asarray(tri_idx)
    bary_weights = np.asarray(bary_weights)

    ti = np.zeros((M_PAD, 3), np.int32)
    ti[:M] = tri_idx
    w = np.zeros((M_PAD, 3), np.float32)
    w[:M] = bary_weights

    f_h = _prep_f(f_values)
    masks = np.zeros((2, 128), np.float32)
    masks[0, :64] = 1.0
    masks[1, 64:] = 1.0
    in_maps = []
    for c in range(NCORES):
        sl = slice(c * M_LOC, (c + 1) * M_LOC)
        idx_h, wts_h = _prep_core_inputs(ti[sl], w[sl])
        in_maps.append({"f": f_h, "idx": idx_h, "wts": wts_h, "masks": masks})

    nc = build_nc()
    res = run_bass_kernel_spmd(nc, in_maps, core_ids=list(range(NCORES)))
    out = np.concatenate([res.results[c]["out"] for c in range(NCORES)], axis=1)
    return out[:, :M].astype(np.float32)


if __name__ == "__main__":
    rng = np.random.default_rng(0)
    f = rng.standard_normal((B, N), dtype=np.float32)
    t_idx = rng.integers(0, N, size=(M, 3)).astype(np.int32)
    bw = rng.random((M, 3), dtype=np.float32)
    bw /= bw.sum(1, keepdims=True)
    got = kernel(f, t_idx, bw)
    exp = np.einsum("bmk,mk->bm", f[:, t_idx], bw)
    err = np.abs(got - exp).max() / np.abs(exp).max()
    print("rel err:", err)


# revision 15
# speedup vs baseline: 1.4754x; 1.4110x over previous
"""Barycentric interpolation kernel for Trainium2 (8 NeuronCores).

Problem: out[b, m] = sum_k f_values[b, tri_idx[m, k]] * bary_weights[m, k]
  f_values: (128, 10000) fp32, tri_idx: (500000, 3) int32, bary: (500000, 3) fp32
  out: (128, 500000) fp32

Strategy (per core, M sharded 8 ways):
  - Batches are packed in fp16 PAIRS: SBUF partition p holds batches (2p, 2p+1)
    as one 32-bit lane value, so 64 partitions cover all 128 batches. f is
    duplicated into partitions 64-127, and the two GPSIMD core groups
    (Q7 cores 0-3 = partitions 0-63, cores 4-7 = 64-127) are given DIFFERENT
    index lists — each core iterates only half the indices per tile, halving
    gather time (GPSIMD is the bottleneck engine).
  - Per m-tile of T=2048 points: partitions 0-63 gather the 3*1024 indices of
    the first 1024 points (k-planar), partitions 64-127 the second 1024.
  - Weights are broadcast to the right partition halves with one K=2 TensorE
    matmul per 512-chunk (lhsT = [half masks], rhs = [wA_dup; wB_dup]) into
    PSUM, then ScalarE casts fp32->fp16 into SBUF.
  - VectorE: 3x tensor_mul + 2x tensor_add, fp16 at 2x perf mode.
  - fp16 result (batch-pair interleaved) is de-interleaved by the store DMA
    access pattern; host upcasts to fp32.
"""

import numpy as np
from contextlib import ExitStack

B = 128
N = 10000
M = 500000
NCORES = 8
T = 3072                       # target points per tile
H = T // 2                     # points per partition-half
TILES_PER_CORE = 21
M_LOC = T * TILES_PER_CORE     # 64512 padded points per core
M_PAD = M_LOC * NCORES         # 516096


def _split_drain_waits(nc, mybir):
    """walrus in this toolchain accepts at most one sync-wait on InstDrain;
    move extra waits onto no-ops inserted right after the drain."""
    for f in nc.m.functions:
        for bb in f.blocks:
            insts = list(bb.instructions)
            out, changed = [], False
            for inst in insts:
                out.append(inst)
                si = inst.sync_info
                if (
                    type(inst).__name__ == "InstDrain"
                    and si is not None
                    and si.on_wait is not None
                    and len(si.on_wait) > 1
                ):
                    extras = list(si.on_wait[1:])
                    si.on_wait = [si.on_wait[0]]
                    for w in extras:
                        out.append(
                            mybir.InstNoOp(
                                name=nc.get_next_instruction_name(),
                                engine=inst.engine,
                                sync_info=mybir.SyncInfo(on_wait=[w], on_update=[]),
                                bass_nofuse=True,
                            )
                        )
                    changed = True
            if changed:
                bb.instructions = out


def build_nc(tiles_per_core=TILES_PER_CORE, t=T):
    import concourse.bacc as bacc
    import concourse.tile as tile
    import concourse.mybir as mybir

    fp16 = mybir.dt.float16
    fp32 = mybir.dt.float32
    i16 = mybir.dt.int16

    h = t // 2          # points per half
    ni = 3 * h          # gather indices per core group per tile
    nc = bacc.Bacc()
    f_d = nc.declare_dram_parameter("f", [128, N], fp32, isOutput=False)
    idx = nc.declare_dram_parameter(
        "idx", [tiles_per_core, 128, ni // 16], i16, isOutput=False
    )
    wts = nc.declare_dram_parameter("wts", [tiles_per_core, 2, 3 * t], fp16,
                                    isOutput=False)
    masks = nc.declare_dram_parameter("masks", [2, 128], fp16, isOutput=False)
    out = nc.declare_dram_parameter(
        "out", [128, tiles_per_core * t], fp16, isOutput=True
    )
    # dst view for de-interleaving stores: row-pair p, batch-parity e, column m
    out_r = out[:].rearrange("(p e) m -> p e m", e=2)

    with ExitStack() as ctx:
        tc = ctx.enter_context(tile.TileContext(nc))
        cpool = ctx.enter_context(tc.tile_pool(name="const", bufs=1))
        f_sb = cpool.tile([128, N, 1], fp32)
        nc.sync.dma_start(f_sb[:, :, 0], f_d[:])
        masks_sb = cpool.tile([2, 128], fp16)
        nc.sync.dma_start(masks_sb[:], masks[:])

        idxp = ctx.enter_context(tc.tile_pool(name="idx", bufs=3))
        wp = ctx.enter_context(tc.tile_pool(name="w", bufs=2))
        gp = ctx.enter_context(tc.tile_pool(name="g", bufs=2))
        wbp = ctx.enter_context(tc.tile_pool(name="wb", bufs=3))
        pp = ctx.enter_context(tc.tile_pool(name="p", bufs=3))
        # (per-tag slots: p0/p1/p2 and wb get independent double-buffers)
        op = ctx.enter_context(tc.tile_pool(name="o", bufs=2))
        psp = ctx.enter_context(tc.tile_pool(name="ps", bufs=4, space="PSUM"))

        for ti in range(tiles_per_core):
            idx_sb = idxp.tile([128, ni // 16], i16, tag="idx")
            nc.sync.dma_start(idx_sb[:], idx[ti])
            w_sb = wp.tile([2, 3 * t], fp16, tag="w")
            nc.sync.dma_start(w_sb[:], wts[ti])

            g3 = gp.tile([128, ni, 1], fp32, tag="g3")
            nc.gpsimd.ap_gather(
                g3[:], f_sb[:], idx_sb[:],
                channels=128, num_elems=N, d=1, num_idxs=ni,
            )
            g16 = g3[:, :, 0].bitcast(fp16)  # [128, 2*ni] batch-pair interleaved

            oute = [
                op.tile([128, h], fp16, name=f"oute{e}", tag=f"o{e}")
                for e in range(2)
            ]
            s01 = op.tile([128, t], fp16, tag="s01")
            for k in range(3):
                wb = wbp.tile([128, t], fp16, tag="wb")
                for cc in range(t // 1024):
                    ps = psp.tile([128, 1024], fp32, tag="ps")
                    for c in range(2):
                        off = k * t + cc * 1024 + c * 512
                        nc.tensor.matmul(
                            ps[:, c * 512 : (c + 1) * 512],
                            masks_sb[:],
                            w_sb[:, off : off + 512],
                            start=True,
                            stop=True,
                        )
                    nc.scalar.copy(wb[:, cc * 1024 : (cc + 1) * 1024], ps[:])
                pk = pp.tile([128, t], fp16, tag="p")
                nc.vector.tensor_mul(pk[:], g16[:, k * t : (k + 1) * t], wb[:])
                if k == 1:
                    nc.vector.tensor_add(s01[:], prev_pk[:], pk[:])  # noqa: F821
                elif k == 2:
                    # final add doubles as the batch-pair de-interleave:
                    # one strided add per parity, contiguous outputs
                    s3 = s01[:].rearrange("p (m e) -> p m e", e=2)
                    p3 = pk[:].rearrange("p (m e) -> p m e", e=2)
                    for e in range(2):
                        nc.vector.tensor_add(oute[e][:], s3[:, :, e], p3[:, :, e])
                prev_pk = pk
            # half hf covers points [ti*t + hf*h, +h)
            for hf in range(2):
                for e in range(2):
                    nc.sync.dma_start(
                        out_r[:, e, ti * t + hf * h : ti * t + (hf + 1) * h],
                        oute[e][64 * hf : 64 * (hf + 1)],
                    )

    nc.finalize()
    _split_drain_waits(nc, mybir)
    return nc


# ---------------------------------------------------------------- host side --


def _prep_f(f_values):
    """(128, N) fp32 -> (128, N) fp32-viewed fp16 batch pairs, duplicated."""
    f16 = f_values.astype(np.float16)                    # (128, N)
    pk = np.empty((64, N, 2), np.float16)
    pk[:, :, 0] = f16[0::2]
    pk[:, :, 1] = f16[1::2]
    packed = pk.reshape(64, 2 * N).view(np.float32)      # (64, N)
    return np.ascontiguousarray(np.concatenate([packed, packed], axis=0))


def _wrap16(lists):
    """(G, n) index lists -> (G, 16, n//16) wrapped: idx j at row j%16."""
    g, n = lists.shape
    return lists.reshape(g, n // 16, 16).transpose(0, 2, 1)


def _prep_core_inputs(ti_core, w_core, tiles_per_core=TILES_PER_CORE, t=T):
    h = t // 2
    ni = 3 * h
    # per tile, half A = points [0, h), half B = [h, t); k-planar lists
    a = ti_core.reshape(tiles_per_core, 2, h, 3).astype(np.int16)
    lists = a.transpose(0, 1, 3, 2).reshape(tiles_per_core * 2, ni)  # [tile*half, 3h]
    wrapped = _wrap16(lists).reshape(tiles_per_core, 2, 16, ni // 16)
    idx = np.empty((tiles_per_core, 128, ni // 16), np.int16)
    idx[:, :64] = np.tile(wrapped[:, 0], (1, 4, 1))
    idx[:, 64:] = np.tile(wrapped[:, 1], (1, 4, 1))
    # wts: (tiles, 2, 3t): row h = k-planar weights of half h, each dup x2
    w = w_core.reshape(tiles_per_core, 2, h, 3).astype(np.float16)
    w = w.transpose(0, 1, 3, 2)                          # [tile, half, k, m]
    wts = np.repeat(w.reshape(tiles_per_core, 2, 3 * h), 2, axis=-1)
    return np.ascontiguousarray(idx), np.ascontiguousarray(wts)


def kernel(f_values, tri_idx, bary_weights):
    from concourse.bass_utils import run_bass_kernel_spmd

    f_values = np.ascontiguousarray(np.asarray(f_values, dtype=np.float32))
    tri_idx = np.asarray(tri_idx)
    bary_weights = np.asarray(bary_weights)

    ti = np.zeros((M_PAD, 3), np.int32)
    ti[:M] = tri_idx
    w = np.zeros((M_PAD, 3), np.float32)
    w[:M] = bary_weights

    f_h = _prep_f(f_values)
    masks = np.zeros((2, 128), np.float16)
    masks[0, :64] = 1.0
    masks[1, 64:] = 1.0
    in_maps = []
    for c in range(NCORES):
        sl = slice(c * M_LOC, (c + 1) * M_LOC)
        idx_h, wts_h = _prep_core_inputs(ti[sl], w[sl])
        in_maps.append({"f": f_h, "idx": idx_h, "wts": wts_h, "masks": masks})

    nc = build_nc()
    res = run_bass_kernel_spmd(nc, in_maps, core_ids=list(range(NCORES)))
    out = np.concatenate([res.results[c]["out"] for c in range(NCORES)], axis=1)
    return out[:, :M].astype(np.float32)


if __name__ == "__main__":
    rng = np.random.default_rng(0)
    f = rng.standard_normal((B, N), dtype=np.float32)
    t_idx = rng.integers(0, N, size=(M, 3)).astype(np.int32)
    bw = rng.random((M, 3), dtype=np.float32)
    bw /= bw.sum(1, keepdims=True)
    got = kernel(f, t_idx, bw)
    exp = np.einsum("bmk,mk->bm", f[:, t_idx], bw)
    err = np.abs(got - exp).max() / np.abs(exp).max()
    print("rel err:", err)


# revision 17
# speedup vs baseline: 1.4881x; 1.0086x over previous
"""Barycentric interpolation kernel for Trainium2 (8 NeuronCores).

Problem: out[b, m] = sum_k f_values[b, tri_idx[m, k]] * bary_weights[m, k]
  f_values: (128, 10000) fp32, tri_idx: (500000, 3) int32, bary: (500000, 3) fp32
  out: (128, 500000) fp32

Strategy (per core, M sharded 8 ways):
  - Batches are packed in fp16 PAIRS: SBUF partition p holds batches (2p, 2p+1)
    as one 32-bit lane value, so 64 partitions cover all 128 batches. f is
    duplicated into partitions 64-127, and the two GPSIMD core groups
    (Q7 cores 0-3 = partitions 0-63, cores 4-7 = 64-127) are given DIFFERENT
    index lists — each core iterates only half the indices per tile, halving
    gather time (GPSIMD is the bottleneck engine).
  - Per m-tile of T points: partitions 0-63 gather the 3*(T/2) indices of
    the first T/2 points (k-planar), partitions 64-127 the second T/2.
  - Weights are broadcast to the right partition halves with one K=2 TensorE
    matmul per 512-chunk (lhsT = [half masks], rhs = [wA_dup; wB_dup]) into
    PSUM, then ScalarE casts fp32->fp16 into SBUF.
  - VectorE: 3x tensor_mul + 2x tensor_add, fp16 at 2x perf mode.
  - fp16 result is stored batch-pair interleaved with one contiguous DMA per
    tile; the host de-interleaves (a fixed unshard permutation) and upcasts.
"""

import numpy as np
from contextlib import ExitStack

B = 128
N = 10000
M = 500000
NCORES = 8
T = 3072                       # target points per tile
H = T // 2                     # points per partition-half
TILES_PER_CORE = 21
M_LOC = T * TILES_PER_CORE     # 64512 padded points per core
M_PAD = M_LOC * NCORES         # 516096


def _split_drain_waits(nc, mybir):
    """walrus in this toolchain accepts at most one sync-wait on InstDrain;
    move extra waits onto no-ops inserted right after the drain."""
    for f in nc.m.functions:
        for bb in f.blocks:
            insts = list(bb.instructions)
            out, changed = [], False
            for inst in insts:
                out.append(inst)
                si = inst.sync_info
                if (
                    type(inst).__name__ == "InstDrain"
                    and si is not None
                    and si.on_wait is not None
                    and len(si.on_wait) > 1
                ):
                    extras = list(si.on_wait[1:])
                    si.on_wait = [si.on_wait[0]]
                    for w in extras:
                        out.append(
                            mybir.InstNoOp(
                                name=nc.get_next_instruction_name(),
                                engine=inst.engine,
                                sync_info=mybir.SyncInfo(on_wait=[w], on_update=[]),
                                bass_nofuse=True,
                            )
                        )
                    changed = True
            if changed:
                bb.instructions = out


def build_nc(tiles_per_core=TILES_PER_CORE, t=T):
    import concourse.bacc as bacc
    import concourse.tile as tile
    import concourse.mybir as mybir

    fp16 = mybir.dt.float16
    fp32 = mybir.dt.float32
    i16 = mybir.dt.int16

    h = t // 2          # points per half
    ni = 3 * h          # gather indices per core group per tile
    nc = bacc.Bacc()
    f_d = nc.declare_dram_parameter("f", [128, N], fp32, isOutput=False)
    idx = nc.declare_dram_parameter(
        "idx", [tiles_per_core, 128, ni // 16], i16, isOutput=False
    )
    wts = nc.declare_dram_parameter("wts", [tiles_per_core, 2, 3 * t], fp16,
                                    isOutput=False)
    masks = nc.declare_dram_parameter("masks", [2, 128], fp16, isOutput=False)
    out = nc.declare_dram_parameter(
        "out", [128, tiles_per_core * t], fp16, isOutput=True
    )

    with ExitStack() as ctx:
        tc = ctx.enter_context(tile.TileContext(nc))
        cpool = ctx.enter_context(tc.tile_pool(name="const", bufs=1))
        f_sb = cpool.tile([128, N, 1], fp32)
        nc.sync.dma_start(f_sb[:, :, 0], f_d[:])
        masks_sb = cpool.tile([2, 128], fp16)
        nc.sync.dma_start(masks_sb[:], masks[:])

        idxp = ctx.enter_context(tc.tile_pool(name="idx", bufs=3))
        wp = ctx.enter_context(tc.tile_pool(name="w", bufs=2))
        gp = ctx.enter_context(tc.tile_pool(name="g", bufs=2))
        wbp = ctx.enter_context(tc.tile_pool(name="wb", bufs=3))
        pp = ctx.enter_context(tc.tile_pool(name="p", bufs=3))
        # (per-tag slots: p0/p1/p2 and wb get independent double-buffers)
        op = ctx.enter_context(tc.tile_pool(name="o", bufs=2))
        psp = ctx.enter_context(tc.tile_pool(name="ps", bufs=4, space="PSUM"))

        for ti in range(tiles_per_core):
            idx_sb = idxp.tile([128, ni // 16], i16, tag="idx")
            nc.sync.dma_start(idx_sb[:], idx[ti])
            w_sb = wp.tile([2, 3 * t], fp16, tag="w")
            nc.sync.dma_start(w_sb[:], wts[ti])

            g3 = gp.tile([128, ni, 1], fp32, tag="g3")
            nc.gpsimd.ap_gather(
                g3[:], f_sb[:], idx_sb[:],
                channels=128, num_elems=N, d=1, num_idxs=ni,
            )
            g16 = g3[:, :, 0].bitcast(fp16)  # [128, 2*ni] batch-pair interleaved

            outt = op.tile([128, t], fp16, tag="o")
            s01 = op.tile([128, t], fp16, tag="s01")
            for k in range(3):
                wb = wbp.tile([128, t], fp16, tag="wb")
                for cc in range(t // 1024):
                    ps = psp.tile([128, 1024], fp32, tag="ps")
                    for c in range(2):
                        off = k * t + cc * 1024 + c * 512
                        nc.tensor.matmul(
                            ps[:, c * 512 : (c + 1) * 512],
                            masks_sb[:],
                            w_sb[:, off : off + 512],
                            start=True,
                            stop=True,
                        )
                    nc.scalar.copy(wb[:, cc * 1024 : (cc + 1) * 1024], ps[:])
                pk = pp.tile([128, t], fp16, tag="p")
                nc.vector.tensor_mul(pk[:], g16[:, k * t : (k + 1) * t], wb[:])
                if k == 1:
                    nc.vector.tensor_add(s01[:], prev_pk[:], pk[:])  # noqa: F821
                elif k == 2:
                    nc.vector.tensor_add(outt[:], s01[:], pk[:])
                prev_pk = pk
            # store batch-pair-interleaved; host de-interleaves (fixed layout)
            nc.sync.dma_start(out[:, ti * t : (ti + 1) * t], outt[:])

    nc.finalize()
    _split_drain_waits(nc, mybir)
    return nc


# ---------------------------------------------------------------- host side --


def _prep_f(f_values):
    """(128, N) fp32 -> (128, N) fp32-viewed fp16 batch pairs, duplicated."""
    f16 = f_values.astype(np.float16)                    # (128, N)
    pk = np.empty((64, N, 2), np.float16)
    pk[:, :, 0] = f16[0::2]
    pk[:, :, 1] = f16[1::2]
    packed = pk.reshape(64, 2 * N).view(np.float32)      # (64, N)
    return np.ascontiguousarray(np.concatenate([packed, packed], axis=0))


def _wrap16(lists):
    """(G, n) index lists -> (G, 16, n//16) wrapped: idx j at row j%16."""
    g, n = lists.shape
    return lists.reshape(g, n // 16, 16).transpose(0, 2, 1)


def _prep_core_inputs(ti_core, w_core, tiles_per_core=TILES_PER_CORE, t=T):
    h = t // 2
    ni = 3 * h
    # per tile, half A = points [0, h), half B = [h, t); k-planar lists
    a = ti_core.reshape(tiles_per_core, 2, h, 3).astype(np.int16)
    lists = a.transpose(0, 1, 3, 2).reshape(tiles_per_core * 2, ni)  # [tile*half, 3h]
    wrapped = _wrap16(lists).reshape(tiles_per_core, 2, 16, ni // 16)
    idx = np.empty((tiles_per_core, 128, ni // 16), np.int16)
    idx[:, :64] = np.tile(wrapped[:, 0], (1, 4, 1))
    idx[:, 64:] = np.tile(wrapped[:, 1], (1, 4, 1))
    # wts: (tiles, 2, 3t): row h = k-planar weights of half h, each dup x2
    w = w_core.reshape(tiles_per_core, 2, h, 3).astype(np.float16)
    w = w.transpose(0, 1, 3, 2)                          # [tile, half, k, m]
    wts = np.repeat(w.reshape(tiles_per_core, 2, 3 * h), 2, axis=-1)
    return np.ascontiguousarray(idx), np.ascontiguousarray(wts)


def _deinterleave(core_out, tiles_per_core=TILES_PER_CORE, t=T):
    """[128, tiles*t] batch-pair-interleaved -> [128 batches, tiles*t points]."""
    h = t // 2
    x = core_out.reshape(2, 64, tiles_per_core, h, 2)   # [hf, pp, ti, m, e]
    x = x.transpose(1, 4, 2, 0, 3)                      # [pp, e, ti, hf, m]
    return x.reshape(128, tiles_per_core * t)


def kernel(f_values, tri_idx, bary_weights):
    from concourse.bass_utils import run_bass_kernel_spmd

    f_values = np.ascontiguousarray(np.asarray(f_values, dtype=np.float32))
    tri_idx = np.asarray(tri_idx)
    bary_weights = np.asarray(bary_weights)

    ti = np.zeros((M_PAD, 3), np.int32)
    ti[:M] = tri_idx
    w = np.zeros((M_PAD, 3), np.float32)
    w[:M] = bary_weights

    f_h = _prep_f(f_values)
    masks = np.zeros((2, 128), np.float16)
    masks[0, :64] = 1.0
    masks[1, 64:] = 1.0
    in_maps = []
    for c in range(NCORES):
        sl = slice(c * M_LOC, (c + 1) * M_LOC)
        idx_h, wts_h = _prep_core_inputs(ti[sl], w[sl])
        in_maps.append({"f": f_h, "idx": idx_h, "wts": wts_h, "masks": masks})

    nc = build_nc()
    res = run_bass_kernel_spmd(nc, in_maps, core_ids=list(range(NCORES)))
    out = np.concatenate(
        [_deinterleave(res.results[c]["out"]) for c in range(NCORES)], axis=1
    )
    return out[:, :M].astype(np.float32)


if __name__ == "__main__":
    rng = np.random.default_rng(0)
    f = rng.standard_normal((B, N), dtype=np.float32)
    t_idx = rng.integers(0, N, size=(M, 3)).astype(np.int32)
    bw = rng.random((M, 3), dtype=np.float32)
    bw /= bw.sum(1, keepdims=True)
    got = kernel(f, t_idx, bw)
    exp = np.einsum("bmk,mk->bm", f[:, t_idx], bw)
    err = np.abs(got - exp).max() / np.abs(exp).max()
    print("rel err:", err)


# revision 18
# speedup vs baseline: 1.8691x; 1.2560x over previous
"""Barycentric interpolation kernel for Trainium2 (8 NeuronCores).

Problem: out[b, m] = sum_k f_values[b, tri_idx[m, k]] * bary_weights[m, k]
  f_values: (128, 10000) fp32, tri_idx: (500000, 3) int32, bary: (500000, 3) fp32
  out: (128, 500000) fp32

Strategy (per core, M sharded 8 ways):
  - Batches are packed in fp16 PAIRS: SBUF partition p holds batches (2p, 2p+1)
    as one 32-bit lane value, so 64 partitions cover all 128 batches. f is
    duplicated into partitions 64-127, and the two GPSIMD core groups
    (Q7 cores 0-3 = partitions 0-63, cores 4-7 = 64-127) are given DIFFERENT
    index lists — each core iterates only half the indices per tile, halving
    gather time (GPSIMD is the bottleneck engine).
  - Per m-tile of T points: partitions 0-63 gather the 3*(T/2) indices of
    the first T/2 points (k-planar), partitions 64-127 the second T/2.
  - Weights are broadcast to the right partition halves with one K=2 TensorE
    matmul per 512-chunk (lhsT = [half masks], rhs = [wA_dup; wB_dup]) into
    PSUM, then ScalarE casts fp32->fp16 into SBUF.
  - VectorE: 3x tensor_mul + 2x tensor_add, fp16 at 2x perf mode.
  - fp16 result is stored batch-pair interleaved with one contiguous DMA per
    tile; the host de-interleaves (a fixed unshard permutation) and upcasts.
"""

import numpy as np
from contextlib import ExitStack

B = 128
N = 10000
M = 500000
NCORES = 8
T = 4096                       # target points per tile
H = T // 2                     # points per partition-half
TILES_PER_CORE = 16
M_LOC = T * TILES_PER_CORE     # 65536 padded points per core
M_PAD = M_LOC * NCORES         # 524288


def _split_drain_waits(nc, mybir):
    """walrus in this toolchain accepts at most one sync-wait on InstDrain;
    move extra waits onto no-ops inserted right after the drain."""
    for f in nc.m.functions:
        for bb in f.blocks:
            insts = list(bb.instructions)
            out, changed = [], False
            for inst in insts:
                out.append(inst)
                si = inst.sync_info
                if (
                    type(inst).__name__ == "InstDrain"
                    and si is not None
                    and si.on_wait is not None
                    and len(si.on_wait) > 1
                ):
                    extras = list(si.on_wait[1:])
                    si.on_wait = [si.on_wait[0]]
                    for w in extras:
                        out.append(
                            mybir.InstNoOp(
                                name=nc.get_next_instruction_name(),
                                engine=inst.engine,
                                sync_info=mybir.SyncInfo(on_wait=[w], on_update=[]),
                                bass_nofuse=True,
                            )
                        )
                    changed = True
            if changed:
                bb.instructions = out


def build_nc(tiles_per_core=TILES_PER_CORE, t=T):
    import concourse.bacc as bacc
    import concourse.tile as tile
    import concourse.mybir as mybir

    fp16 = mybir.dt.float16
    fp32 = mybir.dt.float32
    i16 = mybir.dt.int16

    h = t // 2          # points per half
    ni = 3 * h          # gather indices per core group per tile
    nc = bacc.Bacc()
    f_d = nc.declare_dram_parameter("f", [128, N], fp32, isOutput=False)
    idx = nc.declare_dram_parameter(
        "idx", [tiles_per_core, 128, ni // 16], i16, isOutput=False
    )
    wts = nc.declare_dram_parameter("wts", [tiles_per_core, 2, 3 * t], fp16,
                                    isOutput=False)
    masks = nc.declare_dram_parameter("masks", [2, 128], fp16, isOutput=False)
    out = nc.declare_dram_parameter(
        "out", [128, tiles_per_core * t], fp16, isOutput=True
    )

    with ExitStack() as ctx:
        tc = ctx.enter_context(tile.TileContext(nc))
        cpool = ctx.enter_context(tc.tile_pool(name="const", bufs=1))
        f_sb = cpool.tile([128, N, 1], fp32)
        nc.sync.dma_start(f_sb[:, :, 0], f_d[:])
        masks_sb = cpool.tile([2, 128], fp16)
        nc.sync.dma_start(masks_sb[:], masks[:])

        idxp = ctx.enter_context(tc.tile_pool(name="idx", bufs=2))
        wp = ctx.enter_context(tc.tile_pool(name="w", bufs=2))
        gp = ctx.enter_context(tc.tile_pool(name="g", bufs=2))
        wbp = ctx.enter_context(tc.tile_pool(name="wb", bufs=2))
        pp = ctx.enter_context(tc.tile_pool(name="p", bufs=2))
        # (per-tag slots: p0/p1/p2 and wb get independent double-buffers)
        op = ctx.enter_context(tc.tile_pool(name="o", bufs=1))
        psp = ctx.enter_context(tc.tile_pool(name="ps", bufs=4, space="PSUM"))

        for ti in range(tiles_per_core):
            idx_sb = idxp.tile([128, ni // 16], i16, tag="idx")
            nc.sync.dma_start(idx_sb[:], idx[ti])
            w_sb = wp.tile([2, 3 * t], fp16, tag="w")
            nc.sync.dma_start(w_sb[:], wts[ti])

            g3 = gp.tile([128, ni, 1], fp32, tag="g3")
            nc.gpsimd.ap_gather(
                g3[:], f_sb[:], idx_sb[:],
                channels=128, num_elems=N, d=1, num_idxs=ni,
            )
            g16 = g3[:, :, 0].bitcast(fp16)  # [128, 2*ni] batch-pair interleaved

            outt = op.tile([128, t], fp16, tag="o")
            s01 = op.tile([128, t], fp16, tag="s01")
            for k in range(3):
                wb = wbp.tile([128, t], fp16, tag="wb")
                for cc in range(t // 1024):
                    ps = psp.tile([128, 1024], fp32, tag="ps")
                    for c in range(2):
                        off = k * t + cc * 1024 + c * 512
                        nc.tensor.matmul(
                            ps[:, c * 512 : (c + 1) * 512],
                            masks_sb[:],
                            w_sb[:, off : off + 512],
                            start=True,
                            stop=True,
                        )
                    nc.scalar.copy(wb[:, cc * 1024 : (cc + 1) * 1024], ps[:])
                pk = pp.tile([128, t], fp16, tag="p")
                nc.vector.tensor_mul(pk[:], g16[:, k * t : (k + 1) * t], wb[:])
                if k == 1:
                    nc.vector.tensor_add(s01[:], prev_pk[:], pk[:])  # noqa: F821
                elif k == 2:
                    nc.vector.tensor_add(outt[:], s01[:], pk[:])
                prev_pk = pk
            # store batch-pair-interleaved; host de-interleaves (fixed layout)
            nc.sync.dma_start(out[:, ti * t : (ti + 1) * t], outt[:])

    nc.finalize()
    _split_drain_waits(nc, mybir)
    return nc


# ---------------------------------------------------------------- host side --


def _prep_f(f_values):
    """(128, N) fp32 -> (128, N) fp32-viewed fp16 batch pairs, duplicated."""
    f16 = f_values.astype(np.float16)                    # (128, N)
    pk = np.empty((64, N, 2), np.float16)
    pk[:, :, 0] = f16[0::2]
    pk[:, :, 1] = f16[1::2]
    packed = pk.reshape(64, 2 * N).view(np.float32)      # (64, N)
    return np.ascontiguousarray(np.concatenate([packed, packed], axis=0))


def _wrap16(lists):
    """(G, n) index lists -> (G, 16, n//16) wrapped: idx j at row j%16."""
    g, n = lists.shape
    return lists.reshape(g, n // 16, 16).transpose(0, 2, 1)


def _prep_core_inputs(ti_core, w_core, tiles_per_core=TILES_PER_CORE, t=T):
    h = t // 2
    ni = 3 * h
    # per tile, half A = points [0, h), half B = [h, t); k-planar lists
    a = ti_core.reshape(tiles_per_core, 2, h, 3).astype(np.int16)
    lists = a.transpose(0, 1, 3, 2).reshape(tiles_per_core * 2, ni)  # [tile*half, 3h]
    wrapped = _wrap16(lists).reshape(tiles_per_core, 2, 16, ni // 16)
    idx = np.empty((tiles_per_core, 128, ni // 16), np.int16)
    idx[:, :64] = np.tile(wrapped[:, 0], (1, 4, 1))
    idx[:, 64:] = np.tile(wrapped[:, 1], (1, 4, 1))
    # wts: (tiles, 2, 3t): row h = k-planar weights of half h, each dup x2
    w = w_core.reshape(tiles_per_core, 2, h, 3).astype(np.float16)
    w = w.transpose(0, 1, 3, 2)                          # [tile, half, k, m]
    wts = np.repeat(w.reshape(tiles_per_core, 2, 3 * h), 2, axis=-1)
    return np.ascontiguousarray(idx), np.ascontiguousarray(wts)


def _deinterleave(core_out, tiles_per_core=TILES_PER_CORE, t=T):
    """[128, tiles*t] batch-pair-interleaved -> [128 batches, tiles*t points]."""
    h = t // 2
    x = core_out.reshape(2, 64, tiles_per_core, h, 2)   # [hf, pp, ti, m, e]
    x = x.transpose(1, 4, 2, 0, 3)                      # [pp, e, ti, hf, m]
    return x.reshape(128, tiles_per_core * t)


def kernel(f_values, tri_idx, bary_weights):
    from concourse.bass_utils import run_bass_kernel_spmd

    f_values = np.ascontiguousarray(np.asarray(f_values, dtype=np.float32))
    tri_idx = np.asarray(tri_idx)
    bary_weights = np.asarray(bary_weights)

    ti = np.zeros((M_PAD, 3), np.int32)
    ti[:M] = tri_idx
    w = np.zeros((M_PAD, 3), np.float32)
    w[:M] = bary_weights

    f_h = _prep_f(f_values)
    masks = np.zeros((2, 128), np.float16)
    masks[0, :64] = 1.0
    masks[1, 64:] = 1.0
    in_maps = []
    for c in range(NCORES):
        sl = slice(c * M_LOC, (c + 1) * M_LOC)
        idx_h, wts_h = _prep_core_inputs(ti[sl], w[sl])
        in_maps.append({"f": f_h, "idx": idx_h, "wts": wts_h, "masks": masks})

    nc = build_nc()
    res = run_bass_kernel_spmd(nc, in_maps, core_ids=list(range(NCORES)))
    out = np.concatenate(
        [_deinterleave(res.results[c]["out"]) for c in range(NCORES)], axis=1
    )
    return out[:, :M].astype(np.float32)


if __name__ == "__main__":
    rng = np.random.default_rng(0)
    f = rng.standard_normal((B, N), dtype=np.float32)
    t_idx = rng.integers(0, N, size=(M, 3)).astype(np.int32)
    bw = rng.random((M, 3), dtype=np.float32)
    bw /= bw.sum(1, keepdims=True)
    got = kernel(f, t_idx, bw)
    exp = np.einsum("bmk,mk->bm", f[:, t_idx], bw)
    err = np.abs(got - exp).max() / np.abs(exp).max()
    print("rel err:", err)
